# revision 30
# baseline (speedup 1.0000x reference)
"""Multi-head attention (B=2, S=2048, D=1024, H=16) on 8 trn2 cores.

Sharding: core c handles batch c//4 and heads 4*(c%4) .. 4*(c%4)+3
(tensor-parallel over heads within a batch group of 4 cores).
Each core computes its 4 heads' Q/K/V projections, attention, the
attention-probability output block, and a partial output projection
(row-sharded Wo); the host sums the 4 partial outputs per batch.
"""

import numpy as np

import concourse.bass as bass
import concourse.mybir as mybir
from concourse.bass_utils import run_bass_kernel_spmd
from concourse.masks import make_identity
from concourse.tile import TileContext

F32 = mybir.dt.float32
F32R = mybir.dt.float32r
AF = mybir.ActivationFunctionType
ALU = mybir.AluOpType

B, S, D = 2, 2048, 1024
H, DEPTH = 16, 64
NCORES = 8
HPC = 4          # heads per core
CPC = HPC * DEPTH  # 256 projection cols per core
SCALE = 0.125    # 1/sqrt(DEPTH)


def build_program():
    nc = bass.Bass("TRN2", target_bir_lowering=False, debug=False,
                   num_devices=NCORES)

    xq = nc.dram_tensor("xq", [S, D], F32, kind="ExternalInput")
    xk = nc.dram_tensor("xk", [S, D], F32, kind="ExternalInput")
    xv = nc.dram_tensor("xv", [S, D], F32, kind="ExternalInput")
    wq = nc.dram_tensor("wq", [D, CPC], F32, kind="ExternalInput")
    wk = nc.dram_tensor("wk", [D, CPC], F32, kind="ExternalInput")
    wv = nc.dram_tensor("wv", [D, CPC], F32, kind="ExternalInput")
    bq = nc.dram_tensor("bq", [CPC], F32, kind="ExternalInput")
    bk = nc.dram_tensor("bk", [CPC], F32, kind="ExternalInput")
    wo = nc.dram_tensor("wo", [CPC, D], F32, kind="ExternalInput")
    ones_d = nc.dram_tensor("ones1", [1, 64], F32, kind="ExternalInput")

    attn_out = nc.dram_tensor("attn_part", [HPC, S, S], F32,
                              kind="ExternalOutput")
    out_part = nc.dram_tensor("out_part", [S, D], F32, kind="ExternalOutput")

    with TileContext(nc) as tc:
        with (
            tc.tile_pool(name="persist", bufs=1) as persist,
            tc.tile_pool(name="consts", bufs=1) as consts,
        ):
            ident = consts.tile([128, 128], F32)
            make_identity(nc, ident[:])
            # [1,64] ones row for the recip partition-broadcast matmul
            ones1 = consts.tile([1, 64], F32, tag="ones1")
            nc.sync.dma_start(out=ones1[:].bitcast(F32R),
                              in_=ones_d.ap().bitcast(F32R))

            # weights resident in SBUF: [128, dchunk*CPC]
            wq_sb = persist.tile([128, 8 * CPC], F32, tag="wq")
            wk_sb = persist.tile([128, 8 * CPC], F32, tag="wk")
            wv_sb = persist.tile([128, 8 * CPC], F32, tag="wv")
            nc.sync.dma_start(out=wq_sb[:].rearrange("p (c n) -> p c n", c=8).bitcast(F32R), in_=wq.ap().rearrange("(c p) n -> p c n", p=128).bitcast(F32R))
            nc.sync.dma_start(out=wk_sb[:].rearrange("p (c n) -> p c n", c=8).bitcast(F32R), in_=wk.ap().rearrange("(c p) n -> p c n", p=128).bitcast(F32R))
            nc.sync.dma_start(out=wv_sb[:].rearrange("p (c n) -> p c n", c=8).bitcast(F32R), in_=wv.ap().rearrange("(c p) n -> p c n", p=128).bitcast(F32R))
            bq_sb = persist.tile([128, 2], F32, tag="bq")
            bk_sb = persist.tile([128, 2], F32, tag="bk")
            nc.sync.dma_start(out=bq_sb[:], in_=bq.ap().rearrange("(c p) -> p c", p=128))
            nc.sync.dma_start(out=bk_sb[:], in_=bk.ap().rearrange("(c p) -> p c", p=128))

            # persistent activations
            # qT/kT: [128 cols(chunk cc), seq]  (two col-chunks cc=0,1)
            qT = [persist.tile([128, S], F32, tag=f"qT{cc}", name=f"qT{cc}") for cc in range(2)]
            kT = [persist.tile([128, S], F32, tag=f"kT{cc}", name=f"kT{cc}") for cc in range(2)]
            # v natural: [128 seq-part, kc*CPC + col]
            v_sb = persist.tile([128, 16 * CPC], F32, tag="v")
            # ctx^T per head: [64 cols, seq] (partition base 0 each)
            ctxT = [persist.tile([64, S], F32, tag=f"ctxT{h}", name=f"ctxT{h}") for h in range(4)]

            # ---------------- stage 1: transpose + projections ----------
            with (
                tc.tile_pool(name="s1raw", bufs=4) as s1raw,
                tc.tile_pool(name="s1win", bufs=1) as s1win,
                tc.tile_pool(name="s1psum", bufs=2, space="PSUM") as s1psum,
                tc.tile_pool(name="s1psv", bufs=2, space="PSUM") as s1psv,
                tc.tile_pool(name="s1pst", bufs=3, space="PSUM") as s1pst,
            ):
                for st in range(4):  # seq tiles of 512
                    wins = {}
                    for tname, xsrc in (("q", xq), ("k", xk), ("v", xv)):
                        raws = []
                        for s4 in range(4):
                            raw = s1raw.tile([128, D], F32, tag="raw")
                            r0 = st * 512 + s4 * 128
                            nc.sync.dma_start(out=raw[:], in_=xsrc[r0:r0 + 128, :])
                            raws.append(raw)
                        win = s1win.tile([128, 8 * 512], F32, tag=f"win{tname}")
                        wins[tname] = win
                        for dc in range(8):
                            ps_t = s1pst.tile([128, 512], F32, tag="tr")
                            for s4 in range(4):
                                nc.tensor.transpose(
                                    ps_t[:, s4 * 128:(s4 + 1) * 128],
                                    raws[s4][:, dc * 128:(dc + 1) * 128],
                                    ident[:],
                                )
                            nc.vector.tensor_copy(
                                win[:, dc * 512:(dc + 1) * 512].bitcast(F32R), ps_t[:])
                    # projections for this seq-tile
                    for tname, w_sb, b_sb, dstT, scl in (
                        ("q", wq_sb, bq_sb, qT, SCALE),
                        ("k", wk_sb, bk_sb, kT, None),
                    ):
                        win = wins[tname]
                        for cc in range(2):
                            ps = s1psum.tile([128, 512], F32, tag="proj")
                            for dc in range(8):
                                nc.tensor.matmul(
                                    ps[:],
                                    w_sb[:, dc * CPC + cc * 128: dc * CPC + cc * 128 + 128].bitcast(F32R),
                                    win[:, dc * 512:(dc + 1) * 512].bitcast(F32R),
                                    start=(dc == 0), stop=(dc == 7),
                                )
                            dst = dstT[cc][:, st * 512:(st + 1) * 512].bitcast(F32R)
                            if scl is None:
                                nc.vector.tensor_scalar_add(dst, ps[:], b_sb[:, cc:cc + 1])
                            else:
                                nc.vector.tensor_scalar(
                                    dst, ps[:], b_sb[:, cc:cc + 1], scl,
                                    ALU.add, ALU.mult)
                    winv = wins["v"]
                    for s4 in range(4):
                        kc = st * 4 + s4
                        psv = s1psv.tile([128, CPC], F32, tag="projv")
                        for dc in range(8):
                            nc.tensor.matmul(
                                psv[:],
                                winv[:, dc * 512 + s4 * 128: dc * 512 + s4 * 128 + 128].bitcast(F32R),
                                wv_sb[:, dc * CPC:(dc + 1) * CPC].bitcast(F32R),
                                start=(dc == 0), stop=(dc == 7),
                            )
                        nc.vector.tensor_copy(
                            v_sb[:, kc * CPC:(kc + 1) * CPC].bitcast(F32R), psv[:])

            # ---------------- stage 2: attention ------------------------
            with (
                tc.tile_pool(name="s2pt", bufs=3) as s2pt,
                tc.tile_pool(name="s2pn", bufs=3) as s2pn,
                tc.tile_pool(name="s2small", bufs=4) as s2small,
                tc.tile_pool(name="psbig", bufs=2, space="PSUM") as psbig,
                tc.tile_pool(name="psctx", bufs=1, space="PSUM") as psctx,
                tc.tile_pool(name="psbc", bufs=1, space="PSUM") as psbc,
            ):
                for pair in range(2):
                    cc = pair
                    heads = (2 * pair, 2 * pair + 1)
                    for jt in range(4):  # q tiles of 512
                        # ---- phase A: S^T -> exp -> ctx^T accumulation
                        ps_ctx = [psctx.tile([64, 512], F32, tag=f"ctx{hi}",
                                             name=f"ps_ctx{hi}")
                                  for hi in range(2)]
                        for kcg in range(8):
                            ps_st = [psbig.tile([128, 1024], F32, tag="big", name=f"ps_st{_i}")
                                     for _i in range(2)]
                            pt = [s2pt.tile([128, 1024], F32, tag=f"PT{hi}", name=f"pt{hi}")
                                  for hi in range(2)]
                            for i in range(2):
                                kc = 2 * kcg + i
                                for hi in range(2):
                                    p0 = 64 * hi
                                    nc.tensor.matmul(
                                        ps_st[hi][:, i * 512:(i + 1) * 512],
                                        kT[cc][p0:p0 + 64, kc * 128:(kc + 1) * 128].bitcast(F32R),
                                        qT[cc][p0:p0 + 64, jt * 512:(jt + 1) * 512].bitcast(F32R),
                                        start=True, stop=True,
                                    )
                            for hi in range(2):
                                nc.scalar.activation(
                                    pt[hi][:].bitcast(F32R), ps_st[hi][:], AF.Exp)
                            for i in range(2):
                                kc = 2 * kcg + i
                                for hi in range(2):
                                    h = heads[hi]
                                    nc.tensor.matmul(
                                        ps_ctx[hi][:],
                                        v_sb[:, kc * CPC + 64 * h: kc * CPC + 64 * h + 64].bitcast(F32R),
                                        pt[hi][:, i * 512:(i + 1) * 512].bitcast(F32R),
                                        start=(kc == 0), stop=(kc == 15),
                                        skip_group_check=True,
                                    )
                        # ---- phase B: S natural -> exp(+rowsum) -> attn out
                        recip = [s2small.tile([128, 32], F32, tag=f"recip{hi}", name=f"recip{hi}")
                                 for hi in range(2)]
                        for qt in range(4):
                            pn = [s2pn.tile([128, S], F32, tag=f"pn{hi}", name=f"pn{hi}")
                                  for hi in range(2)]
                            acc = [s2small.tile([128, 4], F32, tag=f"acc{hi}", name=f"acc{hi}")
                                   for hi in range(2)]
                            q0 = jt * 512 + qt * 128
                            for kh in range(2):
                                ps_sn = [psbig.tile([128, 1024], F32, tag="big", name=f"ps_sn{_i}")
                                         for _i in range(2)]
                                for i in range(2):
                                    kt = 2 * kh + i
                                    for hi in range(2):
                                        p0 = 64 * hi
                                        nc.tensor.matmul(
                                            ps_sn[hi][:, i * 512:(i + 1) * 512],
                                            qT[cc][p0:p0 + 64, q0:q0 + 128].bitcast(F32R),
                                            kT[cc][p0:p0 + 64, kt * 512:(kt + 1) * 512].bitcast(F32R),
                                            start=True, stop=True,
                                        )
                                for hi in range(2):
                                    nc.scalar.activation(
                                        pn[hi][:, kh * 1024:(kh + 1) * 1024],
                                        ps_sn[hi][:], AF.Exp,
                                        accum_out=acc[hi][:, 2 * kh + 1: 2 * kh + 2])
                            for hi in range(2):
                                # total = acc[1] + acc[3]
                                nc.vector.tensor_add(
                                    acc[hi][:, 0:1], acc[hi][:, 1:2], acc[hi][:, 3:4])
                                nc.vector.reciprocal(
                                    recip[hi][:, qt:qt + 1], acc[hi][:, 0:1])
                                nc.vector.tensor_scalar_mul(
                                    pn[hi][:], pn[hi][:], recip[hi][:, qt:qt + 1])
                                nc.sync.dma_start(
                                    out=attn_out[heads[hi], q0:q0 + 128, :],
                                    in_=pn[hi][:])
                        # ---- scale ctx^T by 1/rowsum and store to SBUF
                        # recip values live as [128 q, qt]; flip to free-axis
                        # layout via DVE 32x32 transposes, then replicate
                        # across partitions 0-63 with a K=1 ones matmul.
                        for hi in range(2):
                            fl = s2small.tile([32, 128], F32, tag=f"fl{hi}")
                            for j in range(4):
                                nc.vector.transpose(
                                    fl[0:32, j * 32:(j + 1) * 32],
                                    recip[hi][j * 32:(j + 1) * 32, 0:32])
                            flat = s2small.tile([1, 512], F32, tag=f"flat{hi}",
                                                name=f"flat{hi}")
                            nc.sync.dma_start(
                                out=flat[0:1, :].rearrange(
                                    "o (a b) -> o a b", a=4).bitcast(F32R),
                                in_=fl[0:4, :].bitcast(F32R))
                            ps_bc = psbc.tile([64, 512], F32, tag="bc")
                            nc.tensor.matmul(
                                ps_bc[:], ones1[:].bitcast(F32R),
                                flat[:].bitcast(F32R), start=True, stop=True)
                            bc_sb = s2small.tile([64, 512], F32, tag="bc_sb")
                            nc.vector.tensor_copy(bc_sb[:], ps_bc[:])
                            nc.vector.tensor_mul(
                                ctxT[heads[hi]][:, jt * 512:(jt + 1) * 512].bitcast(F32R),
                                ps_ctx[hi][:], bc_sb[:])

            # ---------------- stage 3: output projection ----------------
            with (
                tc.tile_pool(name="s3out", bufs=3) as s3out,
                tc.tile_pool(name="s3ps", bufs=4, space="PSUM") as s3ps,
                tc.tile_pool(name="s3w", bufs=1) as s3w,
            ):
                wo_sb = s3w.tile([64, 4 * D], F32, tag="wo")
                nc.sync.dma_start(
                    out=wo_sb[:].rearrange("p (h n) -> p h n", h=4).bitcast(F32R),
                    in_=wo.ap().rearrange("(h p) n -> p h n", p=64).bitcast(F32R))
                for st in range(16):
                    o_sb = s3out.tile([128, D], F32, tag="o")
                    for oc in range(2):
                        ps_o = s3ps.tile([128, 512], F32, tag="po")
                        for h in range(4):
                            nc.tensor.matmul(
                                ps_o[:],
                                ctxT[h][:, st * 128:(st + 1) * 128].bitcast(F32R),
                                wo_sb[:, h * D + oc * 512: h * D + oc * 512 + 512].bitcast(F32R),
                                start=(h == 0), stop=(h == 3),
                            )
                        nc.vector.tensor_copy(o_sb[:, oc * 512:(oc + 1) * 512], ps_o[:])
                    nc.sync.dma_start(out=out_part[st * 128:(st + 1) * 128, :], in_=o_sb[:])

    _split_excess_waits(nc)
    return nc


def _split_excess_waits(nc):
    """This walrus build caps sync waits at 1 per regular instruction
    (2 per EventSemaphore); hoist excess waits onto standalone
    EventSemaphore instructions inserted just before."""
    n_new = 0
    for fn in nc.m.functions:
        for bb in fn.blocks:
            new_list = []
            for inst in bb.instructions:
                si = inst.sync_info
                cap = 2 if isinstance(inst, mybir.InstEventSemaphore) else 1
                if si is not None and si.on_wait and len(si.on_wait) > cap:
                    waits = list(si.on_wait)
                    keep, extra = waits[:cap], waits[cap:]
                    for i in range(0, len(extra), 2):
                        ev = mybir.InstEventSemaphore(
                            name=f"{inst.name}_wsplit{n_new}",
                            engine=inst.engine,
                            ins=[], outs=[],
                            sync_info=mybir.SyncInfo(
                                on_wait=extra[i:i + 2], on_update=[]),
                            bass_nofuse=True,
                        )
                        n_new += 1
                        new_list.append(ev)
                    si.on_wait = keep
                new_list.append(inst)
            bb.instructions[:] = new_list
    return n_new


_ONES = np.ones((1, 64), np.float32)

_NC_CACHE = None


def _get_program():
    global _NC_CACHE
    if _NC_CACHE is None:
        _NC_CACHE = build_program()
    return _NC_CACHE


def kernel(query, key, value, Wq, bq, Wk, bk, Wv, bv, Wo, bo):
    query = np.ascontiguousarray(np.asarray(query, dtype=np.float32))
    key = np.ascontiguousarray(np.asarray(key, dtype=np.float32))
    value = np.ascontiguousarray(np.asarray(value, dtype=np.float32))
    Wq = np.asarray(Wq, np.float32); bq = np.asarray(bq, np.float32)
    Wk = np.asarray(Wk, np.float32); bk = np.asarray(bk, np.float32)
    Wv = np.asarray(Wv, np.float32); bv = np.asarray(bv, np.float32)
    Wo = np.asarray(Wo, np.float32); bo = np.asarray(bo, np.float32)

    nc = _get_program()
    in_maps = []
    for c in range(NCORES):
        b, g = divmod(c, 4)
        cs = slice(g * CPC, (g + 1) * CPC)
        in_maps.append({
            "xq": query[b], "xk": key[b], "xv": value[b],
            "wq": np.ascontiguousarray(Wq[:, cs]),
            "wk": np.ascontiguousarray(Wk[:, cs]),
            "wv": np.ascontiguousarray(Wv[:, cs]),
            "bq": np.ascontiguousarray(bq[cs]),
            "bk": np.ascontiguousarray(bk[cs]),
            "wo": np.ascontiguousarray(Wo[cs, :]),
            "ones1": _ONES,
        })
    res = run_bass_kernel_spmd(nc, in_maps, list(range(NCORES)))

    attn = np.empty((B, H, S, S), np.float32)
    out = np.zeros((B, S, D), np.float32)
    for c in range(NCORES):
        b, g = divmod(c, 4)
        attn[b, g * HPC:(g + 1) * HPC] = res.results[c]["attn_part"]
        out[b] += res.results[c]["out_part"]
    # v-bias folded on host: softmax rows sum to 1, so P @ (V + bv) adds
    # bv per head to ctx, contributing bv @ Wo to the output.
    out += (bv @ Wo + bo)[None, None, :]
    return out, attn


# revision 48
# speedup vs baseline: 258166.3577x; 258166.3577x over previous
"""Multi-head attention (B=2, S=2048, D=1024, H=16) on 8 trn2 cores.

Sharding: core c handles batch c//4 and heads 4*(c%4) .. 4*(c%4)+3
(tensor-parallel over heads within a batch group of 4 cores).
Each core computes its 4 heads' Q/K/V projections, attention, the
attention-probability output block, and a partial output projection
(row-sharded Wo); the host sums the 4 partial outputs per batch.
"""

import numpy as np

import concourse.bass as bass
import concourse.mybir as mybir
from concourse.bass_utils import run_bass_kernel_spmd
from concourse.tile import TileContext

F32 = mybir.dt.float32
F32R = mybir.dt.float32r
AF = mybir.ActivationFunctionType
ALU = mybir.AluOpType

B, S, D = 2, 2048, 1024
H, DEPTH = 16, 64
NCORES = 8
HPC = 4          # heads per core
CPC = HPC * DEPTH  # 256 projection cols per core
SCALE = 0.125    # 1/sqrt(DEPTH)


def build_program():
    nc = bass.Bass("TRN2", target_bir_lowering=False, debug=False,
                   num_devices=NCORES)

    xq = nc.dram_tensor("xq", [S, D], F32, kind="ExternalInput")
    xk = nc.dram_tensor("xk", [S, D], F32, kind="ExternalInput")
    xv = nc.dram_tensor("xv", [S, D], F32, kind="ExternalInput")
    wq = nc.dram_tensor("wq", [D, CPC], F32, kind="ExternalInput")
    wk = nc.dram_tensor("wk", [D, CPC], F32, kind="ExternalInput")
    wv = nc.dram_tensor("wv", [D, CPC], F32, kind="ExternalInput")
    bq = nc.dram_tensor("bq", [CPC], F32, kind="ExternalInput")
    bk = nc.dram_tensor("bk", [CPC], F32, kind="ExternalInput")
    wo = nc.dram_tensor("wo", [CPC, D], F32, kind="ExternalInput")
    ones_d = nc.dram_tensor("ones1", [1, 64], F32, kind="ExternalInput")
    ident_d = nc.dram_tensor("ident", [128, 128], F32, kind="ExternalInput")

    attn_out = nc.dram_tensor("attn_part", [HPC, S, S], F32,
                              kind="ExternalOutput")
    out_part = nc.dram_tensor("out_part", [S, D], F32, kind="ExternalOutput")

    with TileContext(nc) as tc:
        with (
            tc.tile_pool(name="persist", bufs=1) as persist,
            tc.tile_pool(name="consts", bufs=1) as consts,
        ):
            ident = consts.tile([128, 128], F32)
            nc.sync.dma_start(out=ident[:].bitcast(F32R),
                              in_=ident_d.ap().bitcast(F32R))
            # [1,64] ones row for the recip partition-broadcast matmul
            ones1 = consts.tile([1, 64], F32, tag="ones1")
            nc.sync.dma_start(out=ones1[:].bitcast(F32R),
                              in_=ones_d.ap().bitcast(F32R))

            wq_sb = persist.tile([128, 8 * CPC], F32, tag="wq")
            wk_sb = persist.tile([128, 8 * CPC], F32, tag="wk")
            wv_sb = persist.tile([128, 8 * CPC], F32, tag="wv")
            nc.sync.dma_start(out=wq_sb[:].rearrange("p (c n) -> p c n", c=8).bitcast(F32R), in_=wq.ap().rearrange("(c p) n -> p c n", p=128).bitcast(F32R))
            nc.sync.dma_start(out=wk_sb[:].rearrange("p (c n) -> p c n", c=8).bitcast(F32R), in_=wk.ap().rearrange("(c p) n -> p c n", p=128).bitcast(F32R))
            nc.sync.dma_start(out=wv_sb[:].rearrange("p (c n) -> p c n", c=8).bitcast(F32R), in_=wv.ap().rearrange("(c p) n -> p c n", p=128).bitcast(F32R))
            bq_sb = persist.tile([128, 2], F32, tag="bq")
            bk_sb = persist.tile([128, 2], F32, tag="bk")
            nc.sync.dma_start(out=bq_sb[:], in_=bq.ap().rearrange("(c p) -> p c", p=128))
            nc.sync.dma_start(out=bk_sb[:], in_=bk.ap().rearrange("(c p) -> p c", p=128))

            # persistent activations
            # qT/kT: [128 cols(chunk cc), seq]  (two col-chunks cc=0,1)
            qT = [persist.tile([128, S], F32, tag=f"qT{cc}", name=f"qT{cc}") for cc in range(2)]
            kT = [persist.tile([128, S], F32, tag=f"kT{cc}", name=f"kT{cc}") for cc in range(2)]
            # v natural: [128 seq-part, kc*CPC + col]
            v_sb = persist.tile([128, 16 * CPC], F32, tag="v")
            # ctx^T per head: [64 cols, seq] (partition base 0 each)
            ctxT = [persist.tile([64, S], F32, tag=f"ctxT{h}", name=f"ctxT{h}") for h in range(4)]

            # ---------------- stage 1: transpose + projections ----------
            with (
                tc.tile_pool(name="s1raw", bufs=3) as s1raw,
                tc.tile_pool(name="s1win", bufs=1) as s1win,
                tc.tile_pool(name="s1psum", bufs=2, space="PSUM") as s1psum,
                tc.tile_pool(name="s1psv", bufs=2, space="PSUM") as s1psv,
                tc.tile_pool(name="s1pst", bufs=3, space="PSUM") as s1pst,
            ):
                for st in range(4):  # seq tiles of 512
                    wins = {}
                    for tname, xsrc in (("q", xq), ("k", xk), ("v", xv)):
                        raw = s1raw.tile([128, 4 * D], F32, tag="raw")
                        r0 = st * 512
                        nc.sync.dma_start(
                            out=raw[:].rearrange("p (a n) -> p a n", a=4).bitcast(F32R),
                            in_=xsrc[r0:r0 + 512, :].rearrange(
                                "(a p) n -> p a n", p=128).bitcast(F32R))
                        win = s1win.tile([128, 8 * 512], F32, tag=f"win{tname}")
                        wins[tname] = win
                        for dc in range(8):
                            ps_t = s1pst.tile([128, 512], F32, tag="tr")
                            for s4 in range(4):
                                nc.tensor.transpose(
                                    ps_t[:, s4 * 128:(s4 + 1) * 128].bitcast(F32R),
                                    raw[:, s4 * D + dc * 128: s4 * D + (dc + 1) * 128].bitcast(F32R),
                                    ident[:].bitcast(F32R),
                                )
                            nc.vector.tensor_copy(
                                win[:, dc * 512:(dc + 1) * 512].bitcast(F32R), ps_t[:])
                    for tname, w_sb, b_sb, dstT, scl in (
                        ("q", wq_sb, bq_sb, qT, SCALE),
                        ("k", wk_sb, bk_sb, kT, None),
                    ):
                        win = wins[tname]
                        for cc in range(2):
                            ps = s1psum.tile([128, 512], F32, tag="proj")
                            for dc in range(8):
                                nc.tensor.matmul(
                                    ps[:],
                                    w_sb[:, dc * CPC + cc * 128: dc * CPC + cc * 128 + 128].bitcast(F32R),
                                    win[:, dc * 512:(dc + 1) * 512].bitcast(F32R),
                                    start=(dc == 0), stop=(dc == 7),
                                )
                            dst = dstT[cc][:, st * 512:(st + 1) * 512].bitcast(F32R)
                            if scl is None:
                                nc.vector.tensor_scalar_add(dst, ps[:], b_sb[:, cc:cc + 1])
                            else:
                                nc.vector.tensor_scalar(
                                    dst, ps[:], b_sb[:, cc:cc + 1], scl,
                                    ALU.add, ALU.mult)
                    winv = wins["v"]
                    for s4 in range(4):
                        kc = st * 4 + s4
                        psv = s1psv.tile([128, CPC], F32, tag="projv")
                        for dc in range(8):
                            nc.tensor.matmul(
                                psv[:],
                                winv[:, dc * 512 + s4 * 128: dc * 512 + s4 * 128 + 128].bitcast(F32R),
                                wv_sb[:, dc * CPC:(dc + 1) * CPC].bitcast(F32R),
                                start=(dc == 0), stop=(dc == 7),
                            )
                        nc.vector.tensor_copy(
                            v_sb[:, kc * CPC:(kc + 1) * CPC].bitcast(F32R), psv[:])

            # ---------------- stage 2: attention ------------------------
            with (
                tc.tile_pool(name="s2pt", bufs=3) as s2pt,
                tc.tile_pool(name="s2pn", bufs=3) as s2pn,
                tc.tile_pool(name="s2small", bufs=4) as s2small,
                tc.tile_pool(name="s2med", bufs=2) as s2med,
                tc.tile_pool(name="psbig", bufs=3, space="PSUM") as psbig,
                tc.tile_pool(name="psctx", bufs=1, space="PSUM") as psctx,
            ):
                for pair in range(2):
                    for jt in range(4):  # q tiles of 512
                        cc = pair
                        heads = (2 * pair, 2 * pair + 1)
                        # ---- phase A: S^T -> exp -> ctx^T accumulation
                        ps_ctx = [psctx.tile([64, 512], F32, tag=f"ctx{hi}",
                                             name=f"ps_ctx{hi}")
                                  for hi in range(2)]
                        for kcg in range(8):
                            ps_st = [psbig.tile([128, 1024], F32, tag="big", name=f"ps_st{_i}")
                                     for _i in range(2)]
                            pt = [s2pt.tile([128, 1024], F32, tag=f"PT{hi}", name=f"pt{hi}")
                                  for hi in range(2)]
                            for i in range(2):
                                kc = 2 * kcg + i
                                for hi in range(2):
                                    p0 = 64 * hi
                                    nc.tensor.matmul(
                                        ps_st[hi][:, i * 512:(i + 1) * 512],
                                        kT[cc][p0:p0 + 64, kc * 128:(kc + 1) * 128].bitcast(F32R),
                                        qT[cc][p0:p0 + 64, jt * 512:(jt + 1) * 512].bitcast(F32R),
                                        start=True, stop=True,
                                    )
                            for hi in range(2):
                                nc.scalar.activation(
                                    pt[hi][:].bitcast(F32R), ps_st[hi][:], AF.Exp)
                            for i in range(2):
                                kc = 2 * kcg + i
                                for hi in range(2):
                                    h = heads[hi]
                                    nc.tensor.matmul(
                                        ps_ctx[hi][:],
                                        v_sb[:, kc * CPC + 64 * h: kc * CPC + 64 * h + 64].bitcast(F32R),
                                        pt[hi][:, i * 512:(i + 1) * 512].bitcast(F32R),
                                        start=(kc == 0), stop=(kc == 15),
                                        skip_group_check=True,
                                    )
                        # ---- phase B: S natural -> exp(+rowsum) -> attn out
                        recip = [s2small.tile([128, 32], F32, tag=f"recip{hi}", name=f"recip{hi}")
                                 for hi in range(2)]
                        for qt in range(4):
                            pn = [s2pn.tile([128, S], F32, tag=f"pn{hi}", name=f"pn{hi}")
                                  for hi in range(2)]
                            acc = [s2small.tile([128, 4], F32, tag=f"acc{hi}", name=f"acc{hi}")
                                   for hi in range(2)]
                            q0 = jt * 512 + qt * 128
                            for kh in range(2):
                                ps_sn = [psbig.tile([128, 1024], F32, tag="big", name=f"ps_sn{_i}")
                                         for _i in range(2)]
                                for i in range(2):
                                    kt = 2 * kh + i
                                    for hi in range(2):
                                        p0 = 64 * hi
                                        nc.tensor.matmul(
                                            ps_sn[hi][:, i * 512:(i + 1) * 512],
                                            qT[cc][p0:p0 + 64, q0:q0 + 128].bitcast(F32R),
                                            kT[cc][p0:p0 + 64, kt * 512:(kt + 1) * 512].bitcast(F32R),
                                            start=True, stop=True,
                                        )
                                for hi in range(2):
                                    nc.scalar.activation(
                                        pn[hi][:, kh * 1024:(kh + 1) * 1024],
                                        ps_sn[hi][:], AF.Exp,
                                        accum_out=acc[hi][:, 2 * kh + 1: 2 * kh + 2])
                            for hi in range(2):
                                # total = acc[1] + acc[3]
                                nc.vector.tensor_add(
                                    acc[hi][:, 0:1], acc[hi][:, 1:2], acc[hi][:, 3:4])
                                nc.vector.reciprocal(
                                    recip[hi][:, qt:qt + 1], acc[hi][:, 0:1])
                                nc.vector.tensor_scalar_mul(
                                    pn[hi][:], pn[hi][:], recip[hi][:, qt:qt + 1])
                                nc.sync.dma_start(
                                    out=attn_out[heads[hi], q0:q0 + 128, :],
                                    in_=pn[hi][:])
                        # ---- scale ctx^T by 1/rowsum and store to SBUF
                        # recip values live as [128 q, qt]; flip to free-axis
                        # layout via DVE 32x32 transposes, then replicate
                        # across partitions 0-63 with a K=1 ones matmul.
                        for hi in range(2):
                            fl = s2small.tile([32, 128], F32, tag=f"fl{hi}")
                            for j in range(4):
                                nc.vector.transpose(
                                    fl[0:32, j * 32:(j + 1) * 32],
                                    recip[hi][j * 32:(j + 1) * 32, 0:32])
                            flat = s2small.tile([1, 512], F32, tag=f"flat{hi}",
                                                name=f"flat{hi}")
                            nc.sync.dma_start(
                                out=flat[0:1, :].rearrange(
                                    "o (a b) -> o a b", a=4).bitcast(F32R),
                                in_=fl[0:4, :].bitcast(F32R))
                            ps_bc = psbig.tile([64, 512], F32, tag="big")
                            nc.tensor.matmul(
                                ps_bc[:], ones1[:].bitcast(F32R),
                                flat[:].bitcast(F32R), start=True, stop=True)
                            bc_sb = s2med.tile([64, 512], F32, tag="bc_sb")
                            nc.vector.tensor_copy(bc_sb[:], ps_bc[:])
                            nc.vector.tensor_mul(
                                ctxT[heads[hi]][:, jt * 512:(jt + 1) * 512].bitcast(F32R),
                                ps_ctx[hi][:], bc_sb[:])

            # ---------------- stage 3: output projection ----------------
            with (
                tc.tile_pool(name="s3out", bufs=3) as s3out,
                tc.tile_pool(name="s3ps", bufs=4, space="PSUM") as s3ps,
                tc.tile_pool(name="s3w", bufs=1) as s3w,
            ):
                wo_sb = s3w.tile([64, 4 * D], F32, tag="wo")
                nc.sync.dma_start(
                    out=wo_sb[:].rearrange("p (h n) -> p h n", h=4).bitcast(F32R),
                    in_=wo.ap().rearrange("(h p) n -> p h n", p=64).bitcast(F32R))
                for st in range(16):
                    o_sb = s3out.tile([128, D], F32, tag="o")
                    for oc in range(2):
                        ps_o = s3ps.tile([128, 512], F32, tag="po")
                        for h in range(4):
                            nc.tensor.matmul(
                                ps_o[:],
                                ctxT[h][:, st * 128:(st + 1) * 128].bitcast(F32R),
                                wo_sb[:, h * D + oc * 512: h * D + oc * 512 + 512].bitcast(F32R),
                                start=(h == 0), stop=(h == 3),
                            )
                        nc.vector.tensor_copy(o_sb[:, oc * 512:(oc + 1) * 512], ps_o[:])
                    nc.sync.dma_start(out=out_part[st * 128:(st + 1) * 128, :], in_=o_sb[:])

    _split_excess_waits(nc)
    return nc


def _split_excess_waits(nc):
    """This walrus build caps sync waits at 1 per regular instruction
    (2 per EventSemaphore); hoist excess waits onto standalone
    EventSemaphore instructions inserted just before."""
    n_new = 0
    for fn in nc.m.functions:
        for bb in fn.blocks:
            new_list = []
            for inst in bb.instructions:
                si = inst.sync_info
                cap = 2 if isinstance(inst, mybir.InstEventSemaphore) else 1
                if si is not None and si.on_wait and len(si.on_wait) > cap:
                    waits = list(si.on_wait)
                    keep, extra = waits[:cap], waits[cap:]
                    for i in range(0, len(extra), 2):
                        ev = mybir.InstEventSemaphore(
                            name=f"{inst.name}_wsplit{n_new}",
                            engine=inst.engine,
                            ins=[], outs=[],
                            sync_info=mybir.SyncInfo(
                                on_wait=extra[i:i + 2], on_update=[]),
                            bass_nofuse=True,
                        )
                        n_new += 1
                        new_list.append(ev)
                    si.on_wait = keep
                new_list.append(inst)
            bb.instructions[:] = new_list
    return n_new


_ONES = np.ones((1, 64), np.float32)
_IDENT = np.eye(128, dtype=np.float32)

_NC_CACHE = None


def _get_program():
    global _NC_CACHE
    if _NC_CACHE is None:
        _NC_CACHE = build_program()
    return _NC_CACHE


def kernel(query, key, value, Wq, bq, Wk, bk, Wv, bv, Wo, bo):
    query = np.ascontiguousarray(np.asarray(query, dtype=np.float32))
    key = np.ascontiguousarray(np.asarray(key, dtype=np.float32))
    value = np.ascontiguousarray(np.asarray(value, dtype=np.float32))
    Wq = np.asarray(Wq, np.float32); bq = np.asarray(bq, np.float32)
    Wk = np.asarray(Wk, np.float32); bk = np.asarray(bk, np.float32)
    Wv = np.asarray(Wv, np.float32); bv = np.asarray(bv, np.float32)
    Wo = np.asarray(Wo, np.float32); bo = np.asarray(bo, np.float32)

    nc = _get_program()
    in_maps = []
    for c in range(NCORES):
        b, g = divmod(c, 4)
        cs = slice(g * CPC, (g + 1) * CPC)
        in_maps.append({
            "xq": query[b], "xk": key[b], "xv": value[b],
            "wq": np.ascontiguousarray(Wq[:, cs]),
            "wk": np.ascontiguousarray(Wk[:, cs]),
            "wv": np.ascontiguousarray(Wv[:, cs]),
            "bq": np.ascontiguousarray(bq[cs]),
            "bk": np.ascontiguousarray(bk[cs]),
            "wo": np.ascontiguousarray(Wo[cs, :]),
            "ones1": _ONES,
            "ident": _IDENT,
        })
    res = run_bass_kernel_spmd(nc, in_maps, list(range(NCORES)))

    attn = np.empty((B, H, S, S), np.float32)
    out = np.zeros((B, S, D), np.float32)
    for c in range(NCORES):
        b, g = divmod(c, 4)
        attn[b, g * HPC:(g + 1) * HPC] = res.results[c]["attn_part"]
        out[b] += res.results[c]["out_part"]
    # v-bias folded on host: softmax rows sum to 1, so P @ (V + bv) adds
    # bv per head to ctx, contributing bv @ Wo to the output.
    out += (bv @ Wo + bo)[None, None, :]
    return out, attn
